# revision 37
# baseline (speedup 1.0000x reference)
"""Trainium2 Bass kernel: depthwise 3x3 stencil conv (SAME, zero-pad) + residual.

Math (per image, per channel):
    out[h,w] = sum_{dh,dw} k[dh,dw] * x[h+dh-1, w+dw-1]  +  x[h,w]

The fixed stencil k = [[1,0,-1],[0,1,0],[-1,0,1]] is rank-2:
    k = outer((1,0,-1),(1,0,-1)) + center(1)
so with t[h,w] = x[h-1,w] - x[h+1,w] (vertical pass):
    out[h,w] = 2*x[h,w] + t[h,w-1] - t[h,w+1]

All-bf16 device pipeline (memory-bound problem: bf16 halves HBM traffic and
doubles DVE throughput via the 2x_1P packed mode):

  - host converts x to bf16; the one-column zero pads of the slab are
    memset once on the DVE at startup (they never come from HBM)
  - device computes out/2 = (beta/2)*x + 0.5*t[w-1] - 0.5*t[w+1]:
      * vertical pass t = V^T @ x on TensorE (bf16 matmul, f32 PSUM)
      * ACT copies PSUM -> SBUF with scale=0.5 and bf16 downcast, in
        2048-column multi-bank reads (amortizes per-instruction overhead)
      * for beta == 2 both horizontal ops are then plain tensor_tensor
        adds/subs on DVE whose all-2-byte operands run at 2 elem/cycle
  - host upconverts and multiplies by 2 (exact: power of two in fp32)

Work is split into 8 units (4 images x 2 w-halves); every unit has its
own xs/ts slab pair in SBUF (16 slabs, ~178 KB/partition) so there is no
buffer reuse and the whole input streams in up front. PE semaphore
increments are per-chunk (not per matmul) so consecutive LDWEIGHTS/
MATMUL pairs pipeline in the PE's reorder window. The last unit's DVE
ops are split per chunk so its store launches right after the last ACT
copy instead of a whole op-pair later.
"""

import sys
import numpy as np

for _p in ("/opt/trn_rl_repo",):
    if _p not in sys.path:
        sys.path.insert(0, _p)

# ---------------- problem constants (hardcoded per contract) ----------------
N_CORES = 8
N, H, W, CH = 32, 112, 112, 96
IMGS_PER_CORE = N // N_CORES          # 4
ROWS = IMGS_PER_CORE * H              # 448 rows per core shard
FS = W * CH                           # 10752 elems per row
PAD = CH                              # one w column of zero padding
MM_N = 512                            # one PSUM bank of fp32
CHUNK = 2048                          # ACT copy width (4 PSUM banks)

WHALF = W // 2                        # 56 output columns per unit
USLAB = (WHALF + 2) * CH              # 5568 slab cols (1 w-col halo each side)
UINT = WHALF * CH                     # 5376 interior cols
NU = IMGS_PER_CORE * 2                # 8 units
N_CH = (USLAB + CHUNK - 1) // CHUNK   # 3 chunks (2048, 2048, 1472)
GP_UNITS = ()                         # GPSIMD offload disabled: its software
                                      # tensor_tensor hammers SBUF and slows
                                      # concurrent DVE ops 3.5x (measured)
HOIST_LDW = False                     # bare (non-self-loading) matmuls after
                                      # a single ldweights fail neuronxcc
                                      # codegen here; keep self-loading MMs

_CACHE = {}
LAST_RESULTS = None  # BassKernelResults of the most recent run (for test.py)


def _build_bass(beta):
    """Raw-bass program with a hand-rolled static schedule, all bf16.

    Per unit u (image u//2, w-half u%2):
        SP :  load DMAs (slab interior only; zero slivers are memset)
        PE :  11 matmuls ps = V^T @ xs[:, g*512:...], sem inc per chunk
        ACT:  3 copies ts[:, chunk] = 0.5 * ps (multi-bank read, bf16 out)
        DVE:  op1  xs[96:5472] += ts[0:5376]           (v = x + t@w-1)
              op2  ts[96:5472] = xs[96:...] - ts[192:] (out/2 = v - t@w+1)
              drain -> inc dve sem (units 0-1 ride ACT chunk-wise; the
              last unit's op2+store run in two halves to cut the tail)
        SP :  store ts[96:5472] -> out rows (SBUF -> HBM)

    PSUM is split into two 4-bank halves psA/psB; chunk c = 3u+j uses
    ps[c%2], so matmuls of chunk c wait for the copy of chunk c-2.
    """
    from concourse import bass, mybir

    bf16 = mybir.dt.bfloat16
    f32 = mybir.dt.float32
    nc = bass.Bass(debug=False)
    x_d = nc.declare_dram_parameter("x", [ROWS, FS], bf16, isOutput=False)
    v_d = nc.declare_dram_parameter("vmat", [H, H], bf16, isOutput=False)
    out_d = nc.declare_dram_parameter("out", [ROWS, FS], bf16, isOutput=True)

    n_g = (USLAB + MM_N - 1) // MM_N      # 11 matmul groups (10x512 + 448)
    CHUNK_G = [(0, 4), (4, 8), (8, n_g)]  # group ranges per chunk

    vt = nc.alloc_sbuf_tensor("vt", [H, H], bf16)
    xs = [nc.alloc_sbuf_tensor(f"xs{k}", [H, USLAB], bf16) for k in range(NU)]
    ts = [nc.alloc_sbuf_tensor(f"ts{k}", [H, USLAB], bf16) for k in range(NU)]
    ps = [nc.alloc_psum_tensor(f"ps{b}", [H, CHUNK], f32) for b in range(2)]

    def unit_rows(u):
        i = u // 2
        return i * H, (i + 1) * H

    # slab <-> x_d column mapping. Even units: slab col s holds x col s-96
    # (sliver s<96 is the zero pad); odd units: slab col s holds x col
    # 5280+s (sliver s>=5472 is the zero pad).
    def load_pieces(u):
        if u == 0:
            # two pieces so the PE starts on chunk 0 as early as possible
            return [(96, 2144), (2144, USLAB)]
        # single piece: 11KB-per-partition rows, best DMA efficiency
        return [(96, USLAB)] if u % 2 == 0 else [(0, 5472)]

    def slab_to_x(u, s):
        return s - 96 if u % 2 == 0 else 5280 + s

    # which load piece a PE chunk must wait for (cumulative coverage):
    # the last piece whose end covers the chunk's column range end
    def piece_for_chunk(u, j):
        end = min((j + 1) * CHUNK, USLAB) if j < N_CH - 1 else USLAB
        pieces = load_pieces(u)
        for p, (a, b) in enumerate(pieces):
            if b >= min(end, pieces[-1][1]):
                return p
        return len(pieces) - 1

    # DVE completion counters: units not on GPSIMD inc s_dve cumulatively
    dve_count = {}
    cnt = 0
    for u in range(NU):
        if u not in GP_UNITS:
            cnt += 1
        dve_count[u] = cnt
    gp_count = {u: i + 1 for i, u in enumerate(GP_UNITS)}
    # store issue order: units in expected completion order (GPSIMD units,
    # if any, finish ~8us late, so push them a few slots back)
    store_order = [u for u in range(NU) if u not in GP_UNITS]
    for i, u in enumerate(GP_UNITS):
        store_order.insert(min(3 * (i + 1) + i, len(store_order)), u)

    from contextlib import ExitStack

    with (
        nc.Block(no_gpsimd_drain=True) as block,
        nc.semaphore("s_vt") as s_vt,
        nc.semaphore("s_z") as s_z,
        nc.semaphore("s_pe") as s_pe,
        nc.semaphore("s_act") as s_act,
        nc.semaphore("s_dve") as s_dve,
        nc.semaphore("s_gp") as s_gp,
        nc.semaphore("s_v1") as s_v1,
        nc.semaphore("s_pe0") as s_pe0,
        nc.semaphore("s_act0") as s_act0,
        ExitStack() as _sems,
    ):
        # per-piece DMA completion semaphores (completions of concurrent
        # DMAs can land out of issue order; per-piece sems are exact)
        s_din = [
            [
                _sems.enter_context(nc.semaphore(f"s_din{u}_{p}"))
                for p in range(len(load_pieces(u)))
            ]
            for u in range(NU)
        ]
        s_dout = [_sems.enter_context(nc.semaphore(f"s_dout{u}")) for u in range(NU)]
        s_dout2 = _sems.enter_context(nc.semaphore("s_dout_last2"))

        @block.sync
        def _(sp: bass.BassEngine):
            # unit 0's first piece goes out before the (tiny) vt load so
            # its transfer starts one dispatch slot (~0.7us) earlier
            a0, b0 = load_pieces(0)[0]
            sp.dma_start(
                out=xs[0][:, a0:b0],
                in_=x_d[0:H, slab_to_x(0, a0) : slab_to_x(0, b0)],
            ).then_inc(s_din[0][0], 16)
            sp.dma_start(out=vt[:, :], in_=v_d[:, :]).then_inc(s_vt, 16)
            for u in range(NU):
                r0, r1 = unit_rows(u)
                for p, (a, b) in enumerate(load_pieces(u)):
                    if u == 0 and p == 0:
                        continue
                    xa = slab_to_x(u, a)
                    sp.dma_start(
                        out=xs[u][:, a:b], in_=x_d[r0:r1, xa : xa + (b - a)]
                    ).then_inc(s_din[u][p], 16)
            for u in store_order:
                r0, r1 = unit_rows(u)
                oc0 = (u % 2) * UINT
                if u == NU - 1:
                    # last unit streams out in two halves (DVE incs s_dve
                    # once per half)
                    base = dve_count[u]
                    for h_lo, h_hi, inc in (
                        (0, UINT // 2, 0),
                        (UINT // 2, UINT, 1),
                    ):
                        sp.wait_ge(s_dve, base + inc)
                        sp.dma_start(
                            out=out_d[r0:r1, oc0 + h_lo : oc0 + h_hi],
                            in_=ts[u][:, PAD + h_lo : PAD + h_hi],
                        ).then_inc(s_dout2 if inc else s_dout[u], 16)
                    continue
                if u in GP_UNITS:
                    sp.wait_ge(s_gp, gp_count[u])
                else:
                    sp.wait_ge(s_dve, dve_count[u])
                sp.dma_start(
                    out=out_d[r0:r1, oc0 : oc0 + UINT],
                    in_=ts[u][:, PAD : PAD + UINT],
                ).then_inc(s_dout[u], 16)
            for u in range(NU):
                sp.wait_ge(s_dout[u], 16)
            sp.wait_ge(s_dout2, 16)

        @block.tensor
        def _(pe: bass.BassEngine):
            pe.wait_ge(s_vt, 16)
            pe.wait_ge(s_z, 1)
            if HOIST_LDW:
                # V is the stationary operand of EVERY matmul: load it into
                # the PE array once and emit bare (non-self-loading)
                # InstMatmults, saving ~200ns of serialized LDWEIGHTS per
                # matmul (works for bf16; fp32 would miscompile)
                pe.ldweights(vt[:, :])

            def emit_mm(out_ap, rhs_ap):
                if not HOIST_LDW:
                    return pe.matmul(
                        out=out_ap, lhsT=vt[:, :], rhs=rhs_ap,
                        start=True, stop=True,
                    )
                return pe.add_instruction(
                    mybir.InstMatmult(
                        name=pe.bass.get_next_instruction_name(),
                        replication_resolution=0,
                        replication_shift_amnt=0,
                        replication_num_rows=0,
                        start_tensor_calc=True,
                        stop_tensor_calc=True,
                        ins=[pe.lower_ap(rhs_ap.opt({0}), opt=False)],
                        outs=[pe.lower_ap(out_ap)],
                        perf_mode=None,
                        is_transpose=False,
                        ifmap_quant_offset=None,
                        weights_quant_offset=None,
                        bass_skip_group_check=False,
                        tile_position=(0, 0),
                        tile_size=(128, 128),
                    )
                )

            for u in range(NU):
                last_p = -1

                def _need_piece(p, u=u):
                    # cumulative: wait every not-yet-waited piece up to p
                    nonlocal last_p
                    while last_p < p:
                        last_p += 1
                        pe.wait_ge(s_din[u][last_p], 16)

                for j, (g_lo, g_hi) in enumerate(CHUNK_G):
                    c = u * N_CH + j  # global chunk index
                    _need_piece(piece_for_chunk(u, j))
                    if c >= 2:
                        # psum half reuse: chunk c-2's copy must be done
                        pe.wait_ge(s_act, c - 1)
                    for g in range(g_lo, g_hi):
                        goff = g * MM_N
                        gn = min(MM_N, USLAB - goff)
                        mm = emit_mm(
                            ps[c % 2][
                                0:H, (g - g_lo) * MM_N : (g - g_lo) * MM_N + gn
                            ],
                            xs[u][:, goff : goff + gn],
                        )
                        if g == g_hi - 1:
                            mm.then_inc(s_pe, 1)
                        elif u == 0 and g == 1:
                            # mid-chunk marker: lets ACT start its first
                            # half-copy two matmuls early
                            mm.then_inc(s_pe0, 1)

        @block.scalar
        def _(act: bass.BassEngine):
            # dummy activation: forces the 1.3us ACT_TABLE_LOAD to happen
            # during the idle preamble instead of before the first copy
            act.mul(ts[0][:, 0:2], ps[0][0:H, 0:2], 0.5)
            for u in range(NU):
                for j in range(N_CH):
                    c0 = j * CHUNK
                    csz = min(CHUNK, USLAB - c0)
                    c = u * N_CH + j
                    if u == 0 and j == 0:
                        # very first copy in two halves so the DVE can
                        # start on the first 1024 columns ~2.5us earlier
                        # (s_act numbering is preserved: only the second
                        # half increments it)
                        act.wait_ge(s_pe0, 1)
                        act.mul(
                            ts[0][:, 0:1024], ps[0][0:H, 0:1024], 0.5
                        ).then_inc(s_act0, 1)
                        act.wait_ge(s_pe, 1)
                        act.mul(
                            ts[0][:, 1024:2048], ps[0][0:H, 1024:2048], 0.5
                        ).then_inc(s_act, 1)
                        continue
                    act.wait_ge(s_pe, c + 1)
                    act.mul(
                        ts[u][:, c0 : c0 + csz], ps[c % 2][0:H, 0:csz], 0.5
                    ).then_inc(s_act, 1)

        def op1(eng, u, lo, hi):
            # v[lo:hi) = (beta/2)*x + t'@w-1  over interior cols [lo, hi)
            if beta == 2.0:
                eng.tensor_tensor(
                    out=xs[u][:, PAD + lo : PAD + hi],
                    in0=xs[u][:, PAD + lo : PAD + hi],
                    in1=ts[u][:, lo:hi],
                    op=mybir.AluOpType.add,
                )
            else:
                eng.scalar_tensor_tensor(
                    out=xs[u][:, PAD + lo : PAD + hi],
                    in0=xs[u][:, PAD + lo : PAD + hi],
                    scalar=float(beta) / 2.0,
                    in1=ts[u][:, lo:hi],
                    op0=mybir.AluOpType.mult,
                    op1=mybir.AluOpType.add,
                )

        def op2(eng, u, lo, hi):
            # out/2 [lo:hi) = v - t'@w+1
            eng.tensor_tensor(
                out=ts[u][:, PAD + lo : PAD + hi],
                in0=xs[u][:, PAD + lo : PAD + hi],
                in1=ts[u][:, 2 * PAD + lo : 2 * PAD + hi],
                op=mybir.AluOpType.subtract,
            )

        @block.vector
        def _(dve: bass.BassEngine):
            # zero the pad slivers once; loads never touch them
            for u in range(NU):
                if u % 2 == 0:
                    dve.memset(xs[u][:, 0:PAD], 0.0)
                else:
                    dve.memset(xs[u][:, USLAB - PAD : USLAB], 0.0)
            dve.drain().then_inc(s_z, 1)
            for u in range(NU):
                base_c = u * N_CH
                if u <= 3:
                    # early units are ACT-gated: chunk-split the ops so
                    # DVE rides each ACT copy instead of waiting for all 3.
                    # (s_act >= k implies s_pe >= k since each ACT copy
                    # waits on its PE chunk; explicit s_pe waits are only
                    # needed one chunk AHEAD of the copies.)
                    if u == 0:
                        # ride the half-granular very first copy; the
                        # 1024-piece write [96,1120) only overlaps chunk-0
                        # matmul reads, so s_pe>=1 suffices for it
                        dve.wait_ge(s_act0, 1)
                        dve.wait_ge(s_pe, 1)
                        op1(dve, u, 0, 1024)
                        dve.wait_ge(s_act, 1)
                        dve.wait_ge(s_pe, 2)
                        op1(dve, u, 1024, CHUNK)
                    else:
                        dve.wait_ge(s_act, base_c + 1)
                        dve.wait_ge(s_pe, base_c + 2)
                        op1(dve, u, 0, CHUNK)
                    dve.wait_ge(s_act, base_c + 2)
                    dve.wait_ge(s_pe, base_c + 3)
                    op1(dve, u, CHUNK, 2 * CHUNK)
                    op2(dve, u, 0, CHUNK)
                    dve.wait_ge(s_act, base_c + 3)
                    op1(dve, u, 2 * CHUNK, UINT)
                    op2(dve, u, CHUNK, 2 * CHUNK)
                    op2(dve, u, 2 * CHUNK, UINT)
                    dve.drain().then_inc(s_dve, 1)
                    continue
                # all chunks of unit u must be copied (ts ready); the
                # s_act wait transitively covers the matmul (s_pe) one
                dve.wait_ge(s_act, N_CH * (u + 1))
                op1(dve, u, 0, UINT)
                if u == NU - 1:
                    # last unit: store launches in two halves to cut the
                    # final-store tail
                    op2(dve, u, 0, UINT // 2)
                    dve.drain().then_inc(s_dve, 1)
                    op2(dve, u, UINT // 2, UINT)
                    dve.drain().then_inc(s_dve, 1)
                elif u in GP_UNITS:
                    dve.drain().then_inc(s_v1, 1)
                else:
                    op2(dve, u, 0, UINT)
                    dve.drain().then_inc(s_dve, 1)

        @block.gpsimd
        def _(gp: bass.BassEngine):
            for i, u in enumerate(GP_UNITS):
                gp.wait_ge(s_v1, i + 1)
                op2(gp, u, 0, UINT)
                gp.drain().then_inc(s_gp, 1)

    return nc


def _stencil_params(kern):
    """Validate the depthwise kernel and extract (vertical profile a, beta).

    Requires: channels identical, k[:,2] == -k[:,0], k[0,1] == k[2,1] == 0.
    Returns (a, beta) with a = k[:,0] (vertical mixing profile) and
    beta = k[1,1] + 1 (center coefficient incl. the residual).
    """
    k = np.asarray(kern, dtype=np.float32)
    if k.ndim != 4 or k.shape != (3, 3, 1, CH):
        return None
    if not np.all(k == k[:, :, :, :1]):
        return None
    k2 = k[:, :, 0, 0]
    if not (np.all(k2[:, 2] == -k2[:, 0]) and k2[0, 1] == 0 and k2[2, 1] == 0):
        return None
    return k2[:, 0].copy(), float(k2[1, 1]) + 1.0


def _numpy_fallback(x, kern):
    """Straightforward shifted-add implementation (safety net only)."""
    k = np.asarray(kern, dtype=np.float32)[:, :, 0, :]  # (3,3,CH)
    xp = np.pad(x, ((0, 0), (1, 1), (1, 1), (0, 0)))
    out = x.astype(np.float32).copy()
    for dh in range(3):
        for dw in range(3):
            out += k[dh, dw] * xp[:, dh : dh + H, dw : dw + W, :]
    return out


def _ensure_ntff_hook():
    """The agent image's antenv lacks axon_hooks; synthesize it so
    run_bass_kernel_spmd(trace=True) can reach the NTFF profiler."""
    import types

    if "antenv.axon_hooks" in sys.modules:
        return
    import antenv

    mod = types.ModuleType("antenv.axon_hooks")
    state = {}
    mod.set_axon_ntff_profile_hook = lambda h: state.__setitem__("h", h)
    mod.get_axon_ntff_profile_hook = lambda: state.get("h")
    sys.modules["antenv.axon_hooks"] = mod
    antenv.axon_hooks = mod
    try:
        if "/root/.axon_site" not in sys.path:
            sys.path.insert(0, "/root/.axon_site")
        from trn_agent_boot.trn_boot import _ntff_profile_via_ctypes

        hook = _ntff_profile_via_ctypes("/opt/axon/libaxon_pjrt.so")
        if hook is not None:
            mod.set_axon_ntff_profile_hook(hook)
    except Exception:
        pass


def _run_on_hw(x, a, beta, trace=False):
    global LAST_RESULTS
    if trace:
        _ensure_ntff_hook()
    import ml_dtypes
    from concourse.bass_utils import run_bass_kernel_spmd

    bf16 = ml_dtypes.bfloat16

    # vertical banded matrix: V[i, j] = coeff of x-row i in t-row j
    V = np.zeros((H, H), dtype=np.float32)
    idx = np.arange(H)
    V[idx[:-1] + 1, idx[:-1]] += a[2]   # i = j+1
    V[idx, idx] += a[1]                 # i = j
    V[idx[1:] - 1, idx[1:]] += a[0]     # i = j-1
    Vb = V.astype(bf16)

    key = (a.tobytes(), float(beta))
    if key not in _CACHE:
        _CACHE[key] = _build_bass(float(beta))
    nc = _CACHE[key]

    # host-side bf16 conversion (no padding: device memsets the slivers)
    xb = x.reshape(N_CORES, ROWS, FS).astype(bf16)
    in_maps = [{"x": xb[c], "vmat": Vb} for c in range(N_CORES)]
    res = run_bass_kernel_spmd(nc, in_maps, list(range(N_CORES)), trace=trace)
    LAST_RESULTS = res
    # device returned out/2 in bf16; x2 after upconvert is exact
    out = np.stack(
        [np.asarray(res.results[c]["out"], dtype=np.float32) for c in range(N_CORES)]
    )
    out *= 2.0
    return out.reshape(N, H, W, CH)


def kernel(x, kernel=None, _trace=False, **_unused):
    x = np.ascontiguousarray(np.asarray(x, dtype=np.float32))
    assert x.shape == (N, H, W, CH), f"unexpected x shape {x.shape}"
    if kernel is None:
        base = np.array(
            [[1.0, 0.0, -1.0], [0.0, 1.0, 0.0], [-1.0, 0.0, 1.0]], dtype=np.float32
        )
        kernel = np.tile(base[:, :, None, None], (1, 1, 1, CH))
    params = _stencil_params(kernel)
    if params is None:
        return _numpy_fallback(x, kernel)
    a, beta = params
    return _run_on_hw(x, a, beta, trace=_trace)


if __name__ == "__main__":
    xs = np.random.randn(N, H, W, CH).astype(np.float32)
    out = kernel(xs)
    print(out.shape, out.dtype)
